# revision 33
# baseline (speedup 1.0000x reference)
"""Bilateral filter v3 — TensorE-offloaded weighted tap reduction.
(Verified config: 209,597 ns, rel err 1.931e-04.)
"""
from contextlib import ExitStack

import numpy as np

import concourse.bass as bass
import concourse.bacc as bacc
import concourse.tile as tile
from concourse import mybir

F32 = mybir.dt.float32
F16 = mybir.dt.float16

K = 7
PAD = 3
SQRT50 = float(np.sqrt(50.0))
FULL_H, FULL_W = 480, 640
N_CORES = 8

BR = 18
NP = BR * K
NBLK = 27
SUPER = 3
NSUP = NBLK // SUPER
SROWS = SUPER * BR
PR = 496
PC = 652
C2 = 646
S2 = K * C2


def make_padded(img):
    P = np.zeros((PR, PC), np.float32)
    P[PAD:PAD + FULL_H, PAD:PAD + FULL_W] = img * SQRT50
    return P.astype(np.float16)


def make_weights(g49):
    g2 = np.asarray(g49, np.float32).reshape(K, K)
    lw = np.zeros((NP, K, BR), np.float32)
    for r in range(BR):
        for dx in range(K):
            lw[r * K + dx, :, r] = g2[:, dx]
    return lw.astype(np.float16)


def _v(t, dims, off=0):
    base = t[:]
    return bass.AP(tensor=base.tensor, offset=base.offset + off,
                   ap=[base.ap[0]] + dims)


def emit3(nc, out_ap, img_ap, imgs_ap, imgb_ap, lw_ap, reps=1, hw_loop=False,
          ablate=(), unroll=1):
    DErf = mybir.ActivationFunctionType.Derivative_Erf
    NS = K + 1                  # slot 0 = center row, slots 1..7 = j0..j6
    BSTRIDE = NP * NS * FULL_W

    # contiguous slot ranges per DMA queue; every per-block load is one
    # descriptor per partition
    JSPLIT = [(nc.sync, 0, 3), (nc.gpsimd, 3, 5), (nc.scalar, 5, 8)]

    with tile.TileContext(nc) as tc, ExitStack() as ctx:
        wpool = ctx.enter_context(tc.tile_pool(name="wpool", bufs=1))
        dpool = ctx.enter_context(tc.tile_pool(name="dpool", bufs=4))
        ppool = ctx.enter_context(tc.tile_pool(name="ppool", bufs=2, space="PSUM"))
        opool = ctx.enter_context(tc.tile_pool(name="opool", bufs=2))

        lw = wpool.tile([NP, K, BR], F16)
        nc.sync.dma_start(out=lw, in_=lw_ap)

        for eng, j0, j1 in JSPLIT:
            for dx in range(K):
                eng.dma_start(
                    out=bass.AP(tensor=imgs_ap.tensor,
                                offset=imgs_ap.offset + dx * C2,
                                ap=[[S2, PR], [1, C2]]),
                    in_=bass.AP(tensor=img_ap.tensor,
                                offset=img_ap.offset + dx,
                                ap=[[PC, PR], [1, C2]]),
                )
            for sl in range(j0, j1):
                if sl == 0:
                    # center slot: dx-replicated rows straight from img
                    for b in range(NBLK):
                        eng.dma_start(
                            out=bass.AP(tensor=imgb_ap.tensor,
                                        offset=imgb_ap.offset + b * BSTRIDE,
                                        ap=[[NS * FULL_W, NP], [1, FULL_W]]),
                            in_=bass.AP(tensor=img_ap.tensor,
                                        offset=img_ap.offset
                                        + (18 * b + PAD) * PC + PAD,
                                        ap=[[PC, BR], [0, K], [1, FULL_W]]),
                        )
                    continue
                j = sl - 1
                eng.dma_start(
                    out=bass.AP(tensor=imgb_ap.tensor,
                                offset=imgb_ap.offset + sl * FULL_W,
                                ap=[[BSTRIDE, NBLK], [NS * FULL_W, NP],
                                    [1, FULL_W]]),
                    in_=bass.AP(tensor=imgs_ap.tensor,
                                offset=imgs_ap.offset + j * S2,
                                ap=[[BR * S2, NBLK], [C2, NP], [1, FULL_W]]),
                )

        def do_block_front(b):
            imsh = dpool.tile([NP, NS, FULL_W], F16, name="imsh")
            dt_ = dpool.tile([NP, K, FULL_W], F16, name="dt")
            EU = dpool.tile([NP, K, 2, FULL_W], F16, name="eu")
            if "nodma" not in ablate:
                for eng, j0, j1 in JSPLIT:
                    eng.dma_start(
                        out=imsh[:, j0:j1, :],
                        in_=bass.AP(tensor=imgb_ap.tensor,
                                    offset=imgb_ap.offset + b * BSTRIDE
                                    + j0 * FULL_W,
                                    ap=[[NS * FULL_W, NP],
                                        [1, (j1 - j0) * FULL_W]]),
                    )
            else:
                nc.vector.memset(imsh[:], 0.5)
            if "nodve" not in ablate:
                nc.vector.tensor_tensor(
                    out=dt_[:], in0=imsh[:, 1:NS, :],
                    in1=_v(imsh, [[0, K], [1, FULL_W]]),
                    op=mybir.AluOpType.subtract,
                )
            src = dt_
            if "noact" not in ablate:
                nc.scalar.activation(out=EU[:, :, 0, :], in_=src[:], func=DErf,
                                     bias=0.0, scale=1.0)
            return imsh, EU

        CHUNKS = [(0, 512), (512, 512), (1024, 256)]

        def do_block_back(b, imsh, EU, psum_t):
            s, k = divmod(b, SUPER)
            if "nodve" not in ablate:
                nc.vector.tensor_tensor(
                    out=EU[:, :, 1, :], in0=EU[:, :, 0, :],
                    in1=imsh[:, 1:K + 1, :],
                    op=mybir.AluOpType.mult,
                )
            if "nomm" in ablate:
                return
            for j in range(K):
                lhsT = lw[:, j, :]
                for ci, (off, sz) in enumerate(CHUNKS):
                    nc.tensor.matmul(
                        psum_t[32 * k:32 * k + BR, ci, 0:sz],
                        lhsT,
                        _v(EU, [[1, sz]], off=j * 2 * FULL_W + off),
                        start=(j == 0), stop=(j == K - 1),
                    )

        def do_evac(s, psum_t):
            NE = 64 + BR
            rt = opool.tile([NE, FULL_W], F32, name="rt")
            ob = opool.tile([NE, FULL_W], F32, name="ob")
            nc.vector.reciprocal_approx_fast(
                out=rt[:],
                in_=_v(psum_t, [[1, FULL_W]]),
            )
            nc.vector.scalar_tensor_tensor(
                out=ob[:],
                in0=_v(psum_t, [[1, FULL_W]], off=FULL_W),
                scalar=1.0 / SQRT50,
                in1=rt[:],
                op0=mybir.AluOpType.mult,
                op1=mybir.AluOpType.mult,
            )
            for m in range(SUPER):
                r0 = SROWS * s + BR * m
                rows = min(BR, FULL_H - r0)
                if rows <= 0:
                    break
                nc.gpsimd.dma_start(
                    out=out_ap[r0:r0 + rows, 0:FULL_W],
                    in_=ob[32 * m:32 * m + rows],
                )

        def body():
            LAG = 2
            pend = []
            psum_t = None
            for b in range(NBLK + LAG):
                if b < NBLK:
                    pend.append((b, *do_block_front(b)))
                if len(pend) > LAG or b >= NBLK:
                    b0, imsh0, EU0 = pend.pop(0)
                    s0, k0 = divmod(b0, SUPER)
                    if k0 == 0:
                        psum_t = ppool.tile([64 + BR, 3, 512], F32, name="ps")
                    do_block_back(b0, imsh0, EU0, psum_t)
                    if k0 == SUPER - 1 and "nomm" not in ablate:
                        do_evac(s0, psum_t)

        if hw_loop and reps > 1:
            assert reps % unroll == 0 or unroll == 1
            with tc.For_i(0, reps // unroll, 1):
                for _ in range(unroll):
                    body()
        else:
            for _ in range(reps):
                body()


def build_nc3(reps=1, hw_loop=False, ablate=(), unroll=1):
    nc = bacc.Bacc(num_devices=N_CORES)
    img = nc.dram_tensor("img", [PR, PC], F16, kind="ExternalInput")
    imgs = nc.dram_tensor("imgs", [PR, K, C2], F16, kind="Internal")
    imgb = nc.dram_tensor("imgb", [NBLK, NP, K + 1, FULL_W], F16, kind="Internal")
    lwt = nc.dram_tensor("lwt", [NP, K, BR], F16, kind="ExternalInput")
    out = nc.dram_tensor("out", [FULL_H, FULL_W], F32, kind="ExternalOutput")
    emit3(nc, out.ap(), img.ap(), imgs.ap(), imgb.ap(), lwt.ap(), reps=reps,
          hw_loop=hw_loop, ablate=ablate, unroll=unroll)
    nc.finalize()
    return nc


def make_in_maps(I, g):
    g49 = np.asarray(g, np.float32).reshape(-1)
    lw = make_weights(g49)
    return [{"img": make_padded(np.asarray(I[c, 0], np.float32)), "lwt": lw}
            for c in range(N_CORES)]


def kernel(I: np.ndarray, g: np.ndarray) -> np.ndarray:
    from concourse.bass_utils import run_bass_kernel_spmd

    nc = build_nc3()
    in_maps = make_in_maps(I, g)
    res = run_bass_kernel_spmd(nc, in_maps, core_ids=list(range(N_CORES)))
    global LAST_RESULTS
    LAST_RESULTS = res
    return np.stack([r["out"] for r in res.results], axis=0)


LAST_RESULTS = None
